# revision 5
# baseline (speedup 1.0000x reference)
"""Multi-scale bilinear warp (grid_sample) kernel for Trainium2, 8 NeuronCores.

Problem: 4 scales (S = 512/256/128/64), B=8, C=16.
  out[b,ch,r,c] = bilinear_sample(img[b,ch], y=(c-1)+fy*(S-1)/S, x=(r-1)+fx*(S-1)/S)
  (reference's grid channel convention: channel0 -> x (width coord) built from
   the row index; channel1 -> y (height coord) built from the col index; i.e.
   output is approximately the spatial transpose of img, displaced by the flow.)

Sharding: pure data-parallel, one batch sample per core.

Device algorithm per core/sample, per scale:
  - output is processed in windows of R x C pixels; 8 windows run in parallel,
    one per GPSIMD core group g (16 SBUF partitions each, holding the 16
    channels). A window is processed in n_sub sub-calls of Rs = R/n_sub rows.
  - per-pixel math (sample coords, corner indices, corner weights) runs on the
    vector engine in a "blocked" layout: partition (g,s) holds Npp pixels of
    group g, s in [0,16).
  - the 2x2-corner gather runs on GPSIMD via the ap_gather ucode instruction:
    each group gathers from a per-partition table = img[ch, ywin, xwin] window;
    the int16 index stream of group g is read wrapped across its 16 partitions
    (slot j uses idxs[partition j%16, j//16]), which exactly matches the
    blocked idx tile laid out as [p, 4*f2+k].
  - corner weights are broadcast to the 16 channel partitions of each group by
    a DRAM round-trip (write blocked W4, read back replicated), then a
    multiply + 4-corner reduce on DVE produces the output in a (f2-major, s)
    permuted order per partition; the host unpermutes when unsharding.
"""

import numpy as np

# --------------------------------------------------------------------------
# static problem config
# --------------------------------------------------------------------------
B, CH = 8, 16
SCALES = [512, 256, 128, 64]
NCORES = 8
NG = 8          # gpsimd groups per core
GP = 16         # partitions (channels) per group
P = NG * GP     # 128

MAGIC = np.float32(12582912.0)  # 1.5 * 2**23, round-to-nearest-int trick


def _plan_scale(S, pad):
    """Window plan for one scale. Returns dict of layout constants."""
    if S == 512:
        R, C, n_sub = 64, 64, 2
    elif S == 256:
        R, C, n_sub = 64, 64, 2
    elif S == 128:
        R, C, n_sub = 32, 64, 1
    elif S == 64:
        R, C, n_sub = 8, 64, 1
    else:
        raise ValueError(S)
    nwr, nwc = S // R, S // C
    nwin = nwr * nwc
    assert nwin % NG == 0
    nss = nwin // NG               # supersteps
    Rs = R // n_sub                # rows per sub-call
    Npp = Rs * C // GP             # pixels per partition per call
    xw = min(R + 2 * pad, S)       # table x extent (img x = output row r)
    yw = min(C + 2 * pad, S)       # table y extent (img y = output col c)
    E = yw * xw                    # table elements per partition
    assert E <= 32768, (S, E)
    assert 4 * GP * Npp <= 65535   # num_idxs fits uint16
    return dict(S=S, R=R, C=C, n_sub=n_sub, nwr=nwr, nwc=nwc, nwin=nwin,
                nss=nss, Rs=Rs, Npp=Npp, xw=xw, yw=yw, E=E, pad=pad)


def _rows_cols_f32(S):
    """Bit-exact replicas of the reference's base-grid row/col values."""
    ar = np.arange(S, dtype=np.float32)
    rows = np.float32(-1.0) + (ar - np.float32(1.0)) / np.float32(S - 1) * np.float32(2.0)
    return rows  # cols formula is identical


def _win_base(w0, ext, S):
    """Clamped window base so the window [base, base+ext) covers w0-pad..w0+R+pad."""
    return max(0, min(w0, S - ext))


def _build_plans(flows):
    """Compute pad from actual flow magnitudes (deterministic inputs), build plans."""
    plans = []
    for i, S in enumerate(SCALES):
        f = flows[i]
        m = float(np.abs(f).max()) * (S - 1) / S
        pad = int(np.ceil(m)) + 2
        pad = max(pad, 8)
        plans.append(_plan_scale(S, pad))
    return plans


# --------------------------------------------------------------------------
# host-side data preparation (per core = per batch sample)
# --------------------------------------------------------------------------
def _prep_core(imgs, flows, plans):
    """Build the device input streams for one sample.

    Returns dict with:
      tblsrc: f32 1D   - table data, in exact device load order per superstep
      flw:    f32 [ncalls, 128, 2*Npp_max]  - blocked fx / fy
      cst:    f32 [ncalls, 128, 2*Npp_max]  - rtab / ctab (rows/cols consts)
      pbc:    f32 [ncalls, 128, 1]          - per-group index base offsets
      calls:  python list of per-call metadata (for building the bass program
              and for unpermuting outputs; static given shapes)
    """
    tbl_parts = []
    calls = []
    Npp_max = max(p["Npp"] for p in plans)
    # count calls
    ncalls = sum(p["nss"] * p["n_sub"] for p in plans)
    flw = np.zeros((ncalls, P, 2 * Npp_max), np.float32)
    cst = np.zeros((ncalls, P, 2 * Npp_max), np.float32)
    pbc = np.zeros((ncalls, P, 1), np.float32)

    ci = 0
    out_off = 0
    tbl_off = 0
    for si, pl in enumerate(plans):
        S, R, C = pl["S"], pl["R"], pl["C"]
        Rs, Npp, xw, yw, E = pl["Rs"], pl["Npp"], pl["xw"], pl["yw"], pl["E"]
        pad = pl["pad"]
        img = imgs[si]          # (CH, S, S)
        fx = flows[si][0]       # (S, S)  flow ch0 -> x displacement
        fy = flows[si][1]
        rows = _rows_cols_f32(S)

        for t in range(pl["nss"]):
            # 8 windows for this superstep
            wbases = []
            for g in range(NG):
                w = t * NG + g
                r0 = (w // pl["nwc"]) * R
                c0 = (w % pl["nwc"]) * C
                xb = _win_base(r0 - pad, xw, S)
                yb = _win_base(c0 - pad, yw, S)
                wbases.append((r0, c0, xb, yb))
                # table for this group: img[ch, yb:yb+yw, xb:xb+xw] per channel
            # table tile layout: partition (g,ch) -> flattened window
            tt = np.empty((P, E), np.float32)
            for g in range(NG):
                r0, c0, xb, yb = wbases[g]
                win = img[:, yb:yb + yw, xb:xb + xw]          # (CH, yw, xw)
                tt[g * GP:(g + 1) * GP, :] = win.reshape(CH, E)
            tbl_parts.append(tt.reshape(-1))

            for u in range(pl["n_sub"]):
                # per-call pixel layout: partition (g,s) -> pixels
                # l = s*Npp + f2 in sub-block raster (Rs x C), rows r0+u*Rs+dr
                for g in range(NG):
                    r0, c0, xb, yb = wbases[g]
                    l = np.arange(GP * Npp)
                    dr = l // C
                    dc = l % C
                    r = r0 + u * Rs + dr
                    c = c0 + dc
                    fxv = fx[r, c].astype(np.float32).reshape(GP, Npp)
                    fyv = fy[r, c].astype(np.float32).reshape(GP, Npp)
                    flw[ci, g * GP:(g + 1) * GP, :Npp] = fxv
                    flw[ci, g * GP:(g + 1) * GP, Npp:2 * Npp] = fyv
                    cst[ci, g * GP:(g + 1) * GP, :Npp] = rows[r].reshape(GP, Npp)
                    cst[ci, g * GP:(g + 1) * GP, Npp:2 * Npp] = rows[c].reshape(GP, Npp)
                    pbc[ci, g * GP:(g + 1) * GP, 0] = np.float32(yb * xw + xb)
                calls.append(dict(si=si, t=t, u=u, Npp=Npp, E=E, xw=xw,
                                  tbl_off=tbl_off, out_off=out_off,
                                  wbases=wbases, new_table=(u == 0)))
                out_off += P * GP * Npp
                ci += 1
            tbl_off += P * E
    assert ci == ncalls
    return dict(tblsrc=np.concatenate(tbl_parts), flw=flw, cst=cst, pbc=pbc,
                calls=calls, out_total=out_off)


def _unpermute(outblob, plans, calls):
    """Scatter the per-call output blobs back into (4 scales) full images."""
    outs = [np.empty((CH, p["S"], p["S"]), np.float32) for p in plans]
    for call in calls:
        pl = plans[call["si"]]
        S, C, Rs, Npp = pl["S"], pl["C"], pl["Rs"], pl["Npp"]
        blob = outblob[call["out_off"]:call["out_off"] + P * GP * Npp]
        blob = blob.reshape(P, Npp, GP)          # [(g,ch), f2, s]
        o = outs[call["si"]]
        for g in range(NG):
            r0, c0, xb, yb = call["wbases"][g]
            rbase = r0 + call["u"] * Rs
            # pixel l = s*Npp+f2 ; blob[g*GP+ch, f2, s]
            px = blob[g * GP:(g + 1) * GP]       # (GP, Npp, GP) ch, f2, s
            px = px.transpose(0, 2, 1).reshape(CH, GP * Npp)   # ch, l
            px = px.reshape(CH, Rs, C)
            o[:, rbase:rbase + Rs, c0:c0 + C] = px
    return outs


# --------------------------------------------------------------------------
# golden numpy simulation of the device program (layout-exact)
# --------------------------------------------------------------------------
def _golden_core(imgs, flows, plans, prep):
    HW2 = {}  # per-scale constants
    outblob = np.zeros(prep["out_total"], np.float32)
    tblsrc = prep["tblsrc"]
    for ci, call in enumerate(prep["calls"]):
        pl = plans[call["si"]]
        S, Npp, E, xw = pl["S"], call["Npp"], call["E"], call["xw"]
        q = np.float32(2.0 / S)
        hw = np.float32(0.5 * (S - 1))
        fxv = prep["flw"][ci, :, :Npp]
        fyv = prep["flw"][ci, :, Npp:2 * Npp]
        rtab = prep["cst"][ci, :, :Npp]
        ctab = prep["cst"][ci, :, Npp:2 * Npp]
        bc = prep["pbc"][ci, :, :1]

        # per-pixel math, f32 throughout (mimics device op order)
        t1 = fxv * q + rtab
        x = (t1 + np.float32(1.0)) * hw
        t2 = fyv * q + ctab
        y = (t2 + np.float32(1.0)) * hw
        x0 = (x - np.float32(0.5) + MAGIC) - MAGIC
        y0 = (y - np.float32(0.5) + MAGIC) - MAGIC
        wx1 = x - x0
        wy1 = y - y0
        Sm1 = np.float32(S - 1)
        mx0 = ((x0 >= 0) & (x0 <= Sm1)).astype(np.float32)
        mx1 = ((x0 + 1 >= 0) & (x0 + 1 <= Sm1)).astype(np.float32)
        my0 = ((y0 >= 0) & (y0 <= Sm1)).astype(np.float32)
        my1 = ((y0 + 1 >= 0) & (y0 + 1 <= Sm1)).astype(np.float32)
        vwx0 = (np.float32(1.0) - wx1) * mx0
        vwx1 = wx1 * mx1
        vwy0 = (np.float32(1.0) - wy1) * my0
        vwy1 = wy1 * my1
        xc0 = np.minimum(np.maximum(x0, 0), Sm1)
        yc0 = np.minimum(np.maximum(y0, 0), Sm1)
        xc1 = np.minimum(np.maximum(x0 + np.float32(1.0), 0), Sm1)
        yc1 = np.minimum(np.maximum(y0 + np.float32(1.0), 0), Sm1)
        a0 = yc0 * np.float32(xw) - bc
        a1 = yc1 * np.float32(xw) - bc
        idx = np.empty((P, 4 * Npp), np.float32)
        idx[:, 0::4] = a0 + xc0
        idx[:, 1::4] = a0 + xc1
        idx[:, 2::4] = a1 + xc0
        idx[:, 3::4] = a1 + xc1
        idx16 = idx.astype(np.int16)
        assert (idx16 >= 0).all() and (idx16 < E).all()
        W4 = np.empty((P, 4 * Npp), np.float32)
        W4[:, 0::4] = vwy0 * vwx0
        W4[:, 1::4] = vwy0 * vwx1
        W4[:, 2::4] = vwy1 * vwx0
        W4[:, 3::4] = vwy1 * vwx1

        # gather (ap_gather semantics): per group, stream wrapped across 16 parts
        tt = tblsrc[call["tbl_off"]:call["tbl_off"] + P * E].reshape(P, E)
        dst = np.empty((P, 16 * 4 * Npp), np.float32)
        for g in range(NG):
            rows_i = idx16[g * GP:(g + 1) * GP]        # (16, 4*Npp)
            stream = rows_i.T.reshape(-1)              # slot j -> rows_i[j%16, j//16]
            # wait: wrapped means idx j at (part j%16, free j//16):
            # stream[j] = rows_i[j % 16, j // 16]
            stream = rows_i.T.flatten()                # (f-major, part) -> j=f*16+p? no
            # rows_i.T has shape (4*Npp, 16); flatten gives [f, p] order j = f*16+p ✓
            dst[g * GP:(g + 1) * GP, :] = tt[g * GP:(g + 1) * GP][:, stream]

        # weight broadcast (DRAM round trip): Wb[(g,ch), m] = W4[(g,s),:] blob
        # MAC: dst free pos j = 64*f2 + 16*k + s ; weight at blob m = s*4Npp + 4f2 + k
        f2 = np.arange(Npp)
        k = np.arange(4)
        s = np.arange(GP)
        # build Wb in j-order for each group
        jmap_s = np.empty(16 * 4 * Npp, np.int64)
        j = (64 * f2[:, None, None] + 16 * k[None, :, None] + s[None, None, :])
        m = (s[None, None, :] * 4 * Npp + 4 * f2[:, None, None] + k[None, :, None])
        jmap = np.empty(16 * 4 * Npp, np.int64)
        jmap[j.reshape(-1)] = m.reshape(-1)
        dstW = np.empty_like(dst)
        for g in range(NG):
            blob = W4[g * GP:(g + 1) * GP].reshape(-1)   # (16*4Npp,) s-major
            Wb = blob[jmap]                              # j-order
            dstW[g * GP:(g + 1) * GP] = dst[g * GP:(g + 1) * GP] * Wb[None, :]
        # reduce over k: out[p, f2*16+s] = sum_k dstW[p, 64f2+16k+s]
        d4 = dstW.reshape(P, Npp, 4, GP)
        ob = d4.sum(axis=2, dtype=np.float32)            # (P, Npp, GP) f2-major, s
        outblob[call["out_off"]:call["out_off"] + P * GP * Npp] = ob.reshape(P, -1).reshape(-1)
    return outblob


# --------------------------------------------------------------------------
# bass device program
# --------------------------------------------------------------------------
_BASS_CACHE = {}


def _build_device_program(plans, calls, tbl_total, out_total, Npp_max):
    """Trace + compile the per-core Bass program (same program on all cores)."""
    import concourse.bass as bass
    import concourse.tile as tile
    from concourse import bacc, mybir

    f32 = mybir.dt.float32
    i16 = mybir.dt.int16
    ncalls = len(calls)
    wsc_total = sum(P * 4 * c["Npp"] for c in calls)

    nc = bacc.Bacc("TRN2", target_bir_lowering=False, debug=False,
                   num_devices=NCORES)
    tblsrc = nc.dram_tensor("tblsrc", [tbl_total], f32, kind="ExternalInput").ap()
    flw = nc.dram_tensor("flw", [ncalls, P, 2 * Npp_max], f32, kind="ExternalInput").ap()
    cst = nc.dram_tensor("cst", [ncalls, P, 2 * Npp_max], f32, kind="ExternalInput").ap()
    pbc = nc.dram_tensor("pbc", [ncalls, P, 1], f32, kind="ExternalInput").ap()
    outp = nc.dram_tensor("outp", [out_total], f32, kind="ExternalOutput").ap()
    wscr_h = nc.dram_tensor("wscr", [wsc_total], f32, kind="Internal")
    wscr = wscr_h.ap()

    with tile.TileContext(nc) as tc:
        with (
            tc.tile_pool(name="tblp", bufs=2) as tblp,
            tc.tile_pool(name="dstp", bufs=2) as dstp,
            tc.tile_pool(name="wbp", bufs=1) as wbp,
            tc.tile_pool(name="outp_t", bufs=2) as outpool,
            tc.tile_pool(name="inp", bufs=2) as inp,
            tc.tile_pool(name="tmp", bufs=2) as tmp,
        ):
            tbl_t = None
            woff = 0
            for ci, call in enumerate(calls):
                pl = plans[call["si"]]
                S = pl["S"]
                Npp, E, xw = call["Npp"], call["E"], call["xw"]
                q = float(np.float32(2.0 / S))
                hw = float(np.float32(0.5 * (S - 1)))
                Sm1 = float(S - 1)
                NI = 4 * GP * Npp          # num_idxs per group stream

                if call["new_table"]:
                    tbl_t = tblp.tile([P, E], f32, tag="tbl", name="tbl")
                    src = tblsrc[call["tbl_off"]:call["tbl_off"] + P * E]
                    nc.sync.dma_start(tbl_t[:], src.rearrange("(p e) -> p e", p=P))

                fl = inp.tile([P, 2 * Npp], f32, tag="fl", name="fl")
                nc.sync.dma_start(fl[:], flw[ci, :, :2 * Npp])
                rc = inp.tile([P, 2 * Npp], f32, tag="rc", name="rc")
                nc.sync.dma_start(rc[:], cst[ci, :, :2 * Npp])
                pb = inp.tile([P, 1], f32, tag="pb", name="pb")
                nc.sync.dma_start(pb[:], pbc[ci])

                fx, fy = fl[:, 0:Npp], fl[:, Npp:2 * Npp]
                rtab, ctab = rc[:, 0:Npp], rc[:, Npp:2 * Npp]

                def T(tag):
                    return tmp.tile([P, Npp], f32, tag=tag, name=tag)

                V = nc.vector
                ts, tt = V.tensor_scalar, V.tensor_tensor
                A = mybir.AluOpType

                u = T("u"); ts(u[:], fx, q, None, A.mult)
                t1 = T("t1"); tt(t1[:], u[:], rtab, A.add)
                x = T("x"); ts(x[:], t1[:], 1.0, hw, A.add, A.mult)
                u2 = T("u"); ts(u2[:], fy, q, None, A.mult)
                t2 = T("t1"); tt(t2[:], u2[:], ctab, A.add)
                y = T("y"); ts(y[:], t2[:], 1.0, hw, A.add, A.mult)

                MG = float(MAGIC)
                xm = T("xm"); ts(xm[:], x[:], -0.5, MG, A.add, A.add)
                x0 = T("x0"); ts(x0[:], xm[:], -MG, None, A.add)
                ym = T("ym"); ts(ym[:], y[:], -0.5, MG, A.add, A.add)
                y0 = T("y0"); ts(y0[:], ym[:], -MG, None, A.add)
                wx1 = T("wx1"); tt(wx1[:], x[:], x0[:], A.subtract)
                wy1 = T("wy1"); tt(wy1[:], y[:], y0[:], A.subtract)

                x1 = T("x1"); ts(x1[:], x0[:], 1.0, None, A.add)
                y1 = T("y1"); ts(y1[:], y0[:], 1.0, None, A.add)
                xc0 = T("xc0"); ts(xc0[:], x0[:], 0.0, Sm1, A.max, A.min)
                xc1 = T("xc1"); ts(xc1[:], x1[:], 0.0, Sm1, A.max, A.min)
                yc0 = T("yc0"); ts(yc0[:], y0[:], 0.0, Sm1, A.max, A.min)
                yc1 = T("yc1"); ts(yc1[:], y1[:], 0.0, Sm1, A.max, A.min)
                mx0 = T("mx0"); tt(mx0[:], x0[:], xc0[:], A.is_equal)
                mx1 = T("mx1"); tt(mx1[:], x1[:], xc1[:], A.is_equal)
                my0 = T("my0"); tt(my0[:], y0[:], yc0[:], A.is_equal)
                my1 = T("my1"); tt(my1[:], y1[:], yc1[:], A.is_equal)
                wx0 = T("wx0"); ts(wx0[:], wx1[:], -1.0, 1.0, A.mult, A.add)
                wy0 = T("wy0"); ts(wy0[:], wy1[:], -1.0, 1.0, A.mult, A.add)
                vwx0 = T("vwx0"); tt(vwx0[:], wx0[:], mx0[:], A.mult)
                vwx1 = T("vwx1"); tt(vwx1[:], wx1[:], mx1[:], A.mult)
                vwy0 = T("vwy0"); tt(vwy0[:], wy0[:], my0[:], A.mult)
                vwy1 = T("vwy1"); tt(vwy1[:], wy1[:], my1[:], A.mult)

                a0 = T("a0"); ts(a0[:], yc0[:], float(xw), pb[:], A.mult, A.subtract)
                a1 = T("a1"); ts(a1[:], yc1[:], float(xw), pb[:], A.mult, A.subtract)

                idx4 = tmp.tile([P, 4 * Npp], i16, tag="idx4", name="idx4")
                idx4v = idx4[:].rearrange("p (f k) -> p k f", k=4)
                tt(idx4v[:, 0, :], a0[:], xc0[:], A.add)
                tt(idx4v[:, 1, :], a0[:], xc1[:], A.add)
                tt(idx4v[:, 2, :], a1[:], xc0[:], A.add)
                tt(idx4v[:, 3, :], a1[:], xc1[:], A.add)

                w4 = tmp.tile([P, 4 * Npp], f32, tag="w4", name="w4")
                w4v = w4[:].rearrange("p (f k) -> p k f", k=4)
                tt(w4v[:, 0, :], vwy0[:], vwx0[:], A.mult)
                tt(w4v[:, 1, :], vwy0[:], vwx1[:], A.mult)
                tt(w4v[:, 2, :], vwy1[:], vwx0[:], A.mult)
                tt(w4v[:, 3, :], vwy1[:], vwx1[:], A.mult)

                # weight broadcast round-trip
                nc.sync.dma_start(
                    wscr[woff:woff + P * 4 * Npp].rearrange("(p m) -> p m", p=P),
                    w4[:])
                wb = wbp.tile([P, 16 * 4 * Npp], f32, tag="wb", name="wb")
                wsrc = bass.AP(tensor=wscr_h, offset=woff,
                               ap=[[16 * 4 * Npp, NG], [0, GP], [1, 16 * 4 * Npp]])
                nc.sync.dma_start(wb[:], wsrc)
                woff += P * 4 * Npp

                # gather
                dst = dstp.tile([P, 16 * 4 * Npp], f32, tag="dst", name="dst")
                nc.gpsimd.ap_gather(
                    out_ap=dst[:].rearrange("p (n o) -> p n o", o=1),
                    in_ap=tbl_t[:].rearrange("p (e o) -> p e o", o=1),
                    idxs_ap=idx4[:],
                    channels=P, num_elems=E, d=1, num_idxs=NI)

                # MAC: dst layout j = 64*f2 + 16*k + s ; wb blob m = s*4Npp+4f2+k
                d4 = dst[:].rearrange("p (f k s) -> p f k s", f=Npp, k=4, s=GP)
                wbv = wb[:].rearrange("p (s f k) -> p f k s", s=GP, f=Npp, k=4)
                tt(d4, d4, wbv, A.mult)
                outt = outpool.tile([P, GP * Npp], f32, tag="outt", name="outt")
                V.tensor_reduce(
                    outt[:],
                    dst[:].rearrange("p (f k s) -> p f s k", f=Npp, k=4, s=GP),
                    axis=mybir.AxisListType.X, op=A.add)

                nc.sync.dma_start(
                    outp[call["out_off"]:call["out_off"] + P * GP * Npp]
                    .rearrange("(p m) -> p m", p=P),
                    outt[:])

    nc.compile()
    return nc


# --------------------------------------------------------------------------
# public entry
# --------------------------------------------------------------------------
def kernel(img0, img1, img2, img3, flow0, flow1, flow2, flow3,
           _golden=False, _sim=False):
    imgs_all = [np.asarray(img0), np.asarray(img1), np.asarray(img2), np.asarray(img3)]
    flows_all = [np.asarray(flow0), np.asarray(flow1), np.asarray(flow2), np.asarray(flow3)]
    plans = _build_plans(flows_all)

    outs = [np.empty((B, CH, S, S), np.float32) for S in SCALES]

    if _golden:
        for b in range(B):
            imgs = [a[b] for a in imgs_all]
            flows = [a[b] for a in flows_all]
            prep = _prep_core(imgs, flows, plans)
            blob = _golden_core(imgs, flows, plans, prep)
            res = _unpermute(blob, plans, prep["calls"])
            for i in range(4):
                outs[i][b] = res[i]
        return tuple(outs)

    preps = []
    for b in range(B):
        imgs = [a[b] for a in imgs_all]
        flows = [a[b] for a in flows_all]
        preps.append(_prep_core(imgs, flows, plans))

    p0 = preps[0]
    calls = p0["calls"]
    Npp_max = max(p["Npp"] for p in plans)
    key = tuple(pl["pad"] for pl in plans)
    if key not in _BASS_CACHE:
        _BASS_CACHE[key] = _build_device_program(
            plans, calls, len(p0["tblsrc"]), p0["out_total"], Npp_max)
    nc = _BASS_CACHE[key]

    in_maps = [
        {"tblsrc": pr["tblsrc"], "flw": pr["flw"], "cst": pr["cst"], "pbc": pr["pbc"]}
        for pr in preps
    ]

    if _sim:
        from concourse.bass_interp import CoreSim
        sim = CoreSim(nc)
        for k, v in in_maps[0].items():
            sim.tensor(k)[:] = v
        sim.simulate(check_with_hw=False)
        blobs = [np.array(sim.tensor("outp"))]
        nb = 1
    else:
        from concourse.bass_utils import run_bass_kernel_spmd
        res = run_bass_kernel_spmd(nc, in_maps, core_ids=list(range(NCORES)))
        blobs = [res.results[b]["outp"] for b in range(B)]
        nb = B

    for b in range(nb):
        res_b = _unpermute(blobs[b], plans, preps[b]["calls"])
        for i in range(4):
            outs[i][b] = res_b[i]
    if _sim:
        return tuple(o[:1] for o in outs)
    return tuple(outs)
